# revision 26
# baseline (speedup 1.0000x reference)
"""Trainium2 Bass kernel for nn_AttentionLayer (conv1d -> linear attention -> gelu + residual).

Full inputs:  x [8, 256, 4096] f32, conv_w [512, 256, 3] f32, conv_b [512] f32
Full output:  [8, 256, 4096] f32

Sharding: pure data-parallel over batch B=8 -> 8 NeuronCores, one batch each.
No collectives needed.

Per-core math (C=256, N=4096, one batch):
  y    = conv1d(x, w, pad=1) + b          # [2C, N]
  q    = phi(y[:C]),  k = phi(y[C:])      # phi = elu+1
  v    = x^T                              # [N, C]
  kv   = sum_n phi(k)[n,:] (x) v[n,:]     # [C, C]
  out  = gelu(q @ kv) + x                 # [C, N]

Layout trick: the conv contraction (over input channels ci) produces
q in [c, n] layout (w^T stationary) AND k in [n, c] layout (x stationary)
with zero transposes; all other operands are host-prepped into layouts
where every DMA is large and contiguous-per-partition (15 input DMAs,
sized/ordered so the first conv tile's operands land first — the HWDGE
issue path serializes at ~650ns/DMA and descriptor-heavy transfers are
the classic real-HW cliff, so DMA count/shape is a first-order cost).

phi on the k half uses z = y + b + 1 accumulated directly in PSUM (the
conv bias + 1 enters as a rank-1 matmul appended to each accumulation
group), giving the 2-op form  max(z, min(exp(z-1), 1)):
one ACT exp (bias=-1) + one DVE scalar_tensor_tensor. The q half keeps
the 3-op form with the bias riding the DVE per-partition scalars — the
Q phase is PE-bound with a bias matmul but DVE-bound (and faster)
without one. Matmuls run in bf16 (f32 PSUM accumulate) for pipelined
LDWEIGHTS; dead warmup matmuls + a dummy exp at t~1us start the PE
p-state ramp and the Exp table load before real operands arrive.
Phases run NT -> Q -> KV -> OUT: the ACT table switches Exp->Gelu
exactly once, and KV's pure-PE stretch drains the ACT/DVE backlog
right before the ACT-paced OUT chain. A single 4-slot PSUM pool of
[128,1024] double-bank tiles serves all phases, letting OUT fuse each
gelu across two banks. Residual add and the kv PSUM->SBUF copy run on
DVE (Pool's TensorTensor is ~1.1us/tile and would serialize the OUT
tail; ACT Copy would thrash the activation-table set). Output is bf16
(rel err ~4e-4 of the f32 path, tolerance is 2e-2) to halve the
output DMA bytes; the host casts back to f32.
"""

import ml_dtypes
import numpy as np

import concourse.mybir as mybir
import concourse.tile as tile
from concourse import bacc
from concourse.bass_utils import run_bass_kernel_spmd

F32 = mybir.dt.float32
BF16 = mybir.dt.bfloat16
AF = mybir.ActivationFunctionType
ALU = mybir.AluOpType

B, C, N = 8, 256, 4096
NCORES = 8
CT = C // 128        # 2 c-tiles (partition groups) per 256-channel dim
NJ = N // 512        # 8 column chunks of 512
NT = N // 128        # 32 n-tiles of 128
NP = N + 2           # x padded with one zero column on each side
KW = 3 * CT * 256    # one wt half: 6 blocks of [128, 256]

BF = ml_dtypes.bfloat16


def _build_nc():
    nc = bacc.Bacc("TRN2", target_bir_lowering=False, debug=False, num_devices=NCORES)

    xb_d = nc.declare_dram_parameter("xb", [CT, 128, NP], BF16, isOutput=False)
    vt_d = nc.declare_dram_parameter("vt", [128, NT * 256], BF16, isOutput=False)
    wt_d = nc.declare_dram_parameter("wt", [128, 2 * KW], BF16, isOutput=False)
    bq2_d = nc.declare_dram_parameter("bq2", [128, 5], F32, isOutput=False)
    row_d = nc.declare_dram_parameter("row", [1, 768], BF16, isOutput=False)
    out_d = nc.declare_dram_parameter("out", [C, N], BF16, isOutput=True)

    with tile.TileContext(nc) as tc:
        with (
            tc.tile_pool(name="persist", bufs=1) as per,
            tc.tile_pool(name="tmp", bufs=6) as tmp,
            tc.tile_pool(name="psum", bufs=4, space="PSUM") as ps,
        ):
            # ---- inputs: 11 large DMAs, start-critical ones first --------
            # (xb in three n-slices per ci so the first conv tiles can
            # start while the bulk is still in flight)
            wt_sb = per.tile([128, 2 * KW], BF16, tag="wt")
            xb_sb = [per.tile([128, NP], BF16, tag=f"xb{ci}", name=f"xb{ci}")
                     for ci in range(CT)]
            row_sb = per.tile([1, 768], BF16, tag="row")
            nc.sync.dma_start(out=row_sb, in_=row_d[:, :])
            bq2_sb = per.tile([128, 5], F32, tag="bq2")
            nc.sync.dma_start(out=bq2_sb, in_=bq2_d[:, :])
            nc.sync.dma_start(out=wt_sb[:, 0:256], in_=wt_d[:, 0:256])
            nc.sync.dma_start(out=xb_sb[0][:, 0:514], in_=xb_d[0, :, 0:514])
            nc.sync.dma_start(out=xb_sb[1][:, 0:514], in_=xb_d[1, :, 0:514])
            nc.sync.dma_start(out=wt_sb[:, 256:768], in_=wt_d[:, 256:768])
            nc.sync.dma_start(out=wt_sb[:, 768:1536], in_=wt_d[:, 768:1536])
            for jc in range(3):
                a, b = 514 + jc * 512, 1026 + jc * 512
                for ci in range(CT):
                    nc.sync.dma_start(out=xb_sb[ci][:, a:b],
                                      in_=xb_d[ci, :, a:b])
            for ci in range(CT):
                nc.sync.dma_start(out=xb_sb[ci][:, 2050:NP],
                                  in_=xb_d[ci, :, 2050:NP])
            vt_sb = per.tile([128, NT * 256], BF16, tag="vt")
            nc.sync.dma_start(out=vt_sb, in_=vt_d[:, :])
            nc.sync.dma_start(out=wt_sb[:, KW:2 * KW], in_=wt_d[:, KW:2 * KW])

            onesk = row_sb[0:1, 0:128]         # NT bias matmul stationary
            bk1 = row_sb[0:1, 512:768]         # conv_b[k half] + 1

            def wk(t, cit):                    # k-half weights [128(ci), 256(co)]
                o = (cit * 3 + t) * 256
                return wt_sb[:, o:o + 256]

            def wq(t, cit):                    # q-half weights [128(ci), 256(co)]
                o = KW + (t * CT + cit) * 256
                return wt_sb[:, o:o + 256]

            # ---- persistent intermediates --------------------------------
            kT = per.tile([128, NT, 256], BF16, tag="kT")    # phi(k) in [n, c]
            qphi = [per.tile([128, N], BF16, tag=f"qphi{ct}", name=f"qphi{ct}")
                    for ct in range(CT)]
            kv_sb = per.tile([128, CT, 256], BF16, tag="kv")  # kv in [c, d]

            # ---- warmup: ramp the PE p-state while input DMAs land ------
            # (PE runs at half rate until ~3us of continuous busy; dead
            # matmuls on a memset scratch tile start the ramp at t~0.7us
            # instead of when the first real operands arrive)
            scratch = per.tile([128, 640], BF16, tag="warm")
            nc.vector.memset(scratch, 0.0)
            # dummy 1-elem exp: hoists the Exp table load (1.28us) to t~1us
            # instead of serializing it behind the first real exp's inputs
            dummy = tmp.tile([128, 1], F32, tag="dummy")
            nc.scalar.activation(dummy, scratch[:, 0:1], AF.Exp)
            wm_ps = ps.tile([128, 1024], F32, tag="bank", name="wm_ps")
            for w in range(6):
                nc.tensor.matmul(wm_ps[:, 0:512], scratch[:, 0:128],
                                 scratch[:, 128:640],
                                 start=(w == 0), stop=(w == 5))

            # ---- phase NT: k^T = phi(conv_k + b) in [n, c] layout --------
            for i in range(NT):
                j, off = i // 4, (i % 4) * 128
                kt_ps = ps.tile([128, 1024], F32, tag="bank", name="kt_ps")
                kt_ps = kt_ps[:, 0:256]
                for ci in range(CT):
                    for t in range(3):
                        nc.tensor.matmul(
                            kt_ps,
                            xb_sb[ci][:, j * 512 + off + t:
                                      j * 512 + off + t + 128],
                            wk(t, ci),
                            start=(ci == 0 and t == 0),
                            stop=False,
                        )
                # z = y + (b_k + 1): rank-1 (ones^T @ bk1) ends the group
                nc.tensor.matmul(kt_ps, onesk, bk1, start=False, stop=True)
                # phi = max(z, min(exp(z-1), 1))
                e = tmp.tile([128, 256], F32, tag="nte")
                nc.scalar.activation(e, kt_ps, AF.Exp,
                                 bias=bq2_sb[:, 4:5])
                nc.vector.scalar_tensor_tensor(
                    kT[:, i, :], e, 1.0, kt_ps, ALU.min, ALU.max)

            # ---- phase Q: q = phi(conv_q + b) in [c, n] layout -----------
            for ct in range(CT):
                bq = bq2_sb[:, 2 * ct:2 * ct + 1]
                bq1 = bq2_sb[:, 2 * ct + 1:2 * ct + 2]
                for j in range(NJ):
                    q_ps = ps.tile([128, 1024], F32, tag="bank",
                                   name="q_ps")
                    q_ps = q_ps[:, 0:512]
                    for ci in range(CT):
                        for t in range(3):
                            nc.tensor.matmul(
                                q_ps,
                                wq(t, ci)[:, ct * 128:(ct + 1) * 128],
                                xb_sb[ci][:, j * 512 + t:j * 512 + t + 512],
                                start=(ci == 0 and t == 0),
                                stop=(ci == CT - 1 and t == 2),
                            )
                    # phi: min(y+b, 0) -> exp -> (y + (b+1)) max e
                    # (bias rides the DVE per-partition scalars; cheaper than
                    # a rank-1 bias matmul since Q is otherwise PE-bound)
                    tmin = tmp.tile([128, 512], F32, tag="qtmin")
                    nc.vector.tensor_scalar(
                        tmin, q_ps, bq, 0.0, ALU.add, ALU.min)
                    e = tmp.tile([128, 512], F32, tag="qte")
                    nc.scalar.activation(e, tmin, AF.Exp)
                    nc.vector.scalar_tensor_tensor(
                        qphi[ct][:, j * 512:(j + 1) * 512],
                        q_ps, bq1, e, ALU.add, ALU.max)

            # ---- phase KV: kv[c, d] = sum_n k^T[n, c] v^T[n, d] ----------
            for ch in range(CT):
                kv_ps = ps.tile([128, 1024], F32, tag="bank", name="kv_ps")
                kv_ps = kv_ps[:, 0:256]
                for i in range(NT):
                    nc.tensor.matmul(
                        kv_ps,
                        kT[:, i, ch * 128:(ch + 1) * 128],
                        vt_sb[:, i * 256:(i + 1) * 256],
                        start=(i == 0),
                        stop=(i == NT - 1),
                    )
                nc.vector.tensor_copy(kv_sb[:, ch, :], kv_ps)

            # ---- phase OUT: out[d, n] = gelu(sum_c kv[c, d] q[c, n]) + x -
            # pairs of j-chunks share one residual add + one output DMA so
            # the HWDGE issue rate (625ns/DMA) stays ahead of ACT's gelu
            # rate and the tail drains fast
            for dt in range(CT):
                for jj in range(NJ // 2):
                    last = (dt == CT - 1 and jj == NJ // 2 - 1)
                    o_ps = ps.tile([128, 1024], F32, tag="bank",
                                   name="o_ps")
                    for h in range(2):
                        j = 2 * jj + h
                        for ch in range(CT):
                            nc.tensor.matmul(
                                o_ps[:, h * 512:(h + 1) * 512],
                                kv_sb[:, ch, dt * 128:(dt + 1) * 128],
                                qphi[ch][:, j * 512:(j + 1) * 512],
                                start=(ch == 0),
                                stop=(ch == CT - 1),
                            )
                    if not last:
                        g = tmp.tile([128, 1024], BF16, tag="og")
                        nc.scalar.activation(g, o_ps, AF.Gelu)
                        o = tmp.tile([128, 1024], BF16, tag="oo")
                        nc.vector.tensor_add(
                            o, g,
                            xb_sb[dt][:, 1 + jj * 1024:1 + (jj + 1) * 1024])
                        nc.sync.dma_start(
                            out=out_d[dt * 128:(dt + 1) * 128,
                                      jj * 1024:(jj + 1) * 1024],
                            in_=o,
                        )
                    else:
                        # final group drains per-512 so the tail chain ends
                        # on a half-size gelu/add/DMA
                        for h in range(2):
                            j = 2 * jj + h
                            gh = tmp.tile([128, 512], BF16, tag="ogh")
                            nc.scalar.activation(
                                gh, o_ps[:, h * 512:(h + 1) * 512], AF.Gelu)
                            oh = tmp.tile([128, 512], BF16, tag="ooh")
                            nc.vector.tensor_add(
                                oh, gh,
                                xb_sb[dt][:, 1 + j * 512:1 + (j + 1) * 512])
                            nc.sync.dma_start(
                                out=out_d[dt * 128:(dt + 1) * 128,
                                          j * 512:(j + 1) * 512],
                                in_=oh,
                            )

    nc.compile()
    return nc


_NC_CACHE = None


def _get_nc():
    global _NC_CACHE
    if _NC_CACHE is None:
        _NC_CACHE = _build_nc()
    return _NC_CACHE


def _prep(x, conv_w, conv_b):
    x = np.asarray(x, dtype=np.float32)
    conv_w = np.asarray(conv_w, dtype=np.float32)
    conv_b = np.asarray(conv_b, dtype=np.float32)
    xb = np.zeros((B, CT, 128, NP), dtype=BF)
    xb[:, :, :, 1:N + 1] = x.reshape(B, CT, 128, N).astype(BF)
    # vt[b, p, i*256 + d] = x[b, d, i*128 + p]
    xt = x.transpose(0, 2, 1)                              # [B, N, C]
    vt = np.ascontiguousarray(
        xt.reshape(B, NT, 128, C).transpose(0, 2, 1, 3)
    ).reshape(B, 128, NT * C).astype(BF)
    # wt[ci, half, (t*CT + cit)*256 + co'] = conv_w[half*256 + co', cit*128 + ci, t]
    w4 = (conv_w.transpose(1, 2, 0)                        # [cin, t, co]
          .reshape(CT, 128, 3, 2 * C)                      # [cit, ci, t, co]
          .transpose(1, 2, 0, 3))                          # [ci, t, cit, co]
    wt = np.concatenate(
        [w4[..., C:2 * C].transpose(0, 2, 1, 3)            # k half, cit-major
         .reshape(128, KW),
         w4[..., 0:C].reshape(128, KW)],                   # q half, t-major
        axis=1).astype(BF)
    bq2 = np.empty((128, 5), dtype=np.float32)
    for ct in range(CT):
        bq2[:, 2 * ct] = conv_b[ct * 128:(ct + 1) * 128]
        bq2[:, 2 * ct + 1] = conv_b[ct * 128:(ct + 1) * 128] + 1.0
    bq2[:, 4] = -1.0
    row = np.zeros((1, 768), dtype=np.float32)
    row[0, 0:512] = 1.0
    row[0, 512:768] = conv_b[C:] + 1.0
    return xb, vt, wt, bq2, row.astype(BF)


def make_in_maps(x, conv_w, conv_b):
    xb, vt, wt, bq2, row = _prep(x, conv_w, conv_b)
    return [
        {"xb": xb[b], "vt": vt[b], "wt": wt, "bq2": bq2, "row": row}
        for b in range(B)
    ]


def kernel(x: np.ndarray, conv_w: np.ndarray, conv_b: np.ndarray) -> np.ndarray:
    nc = _get_nc()
    in_maps = make_in_maps(x, conv_w, conv_b)
    res = run_bass_kernel_spmd(nc, in_maps, core_ids=list(range(NCORES)))
    return np.stack(
        [res.results[b]["out"].astype(np.float32) for b in range(B)], axis=0)


# revision 60
# speedup vs baseline: 1.0076x; 1.0076x over previous
"""Trainium2 Bass kernel for nn_AttentionLayer (conv1d -> linear attention -> gelu + residual).

Full inputs:  x [8, 256, 4096] f32, conv_w [512, 256, 3] f32, conv_b [512] f32
Full output:  [8, 256, 4096] f32

Sharding: pure data-parallel over batch B=8 -> 8 NeuronCores, one batch each.
No collectives needed.

Per-core math (C=256, N=4096, one batch):
  y    = conv1d(x, w, pad=1) + b          # [2C, N]
  q    = phi(y[:C]),  k = phi(y[C:])      # phi = elu+1
  v    = x^T                              # [N, C]
  kv   = sum_n phi(k)[n,:] (x) v[n,:]     # [C, C]
  out  = gelu(q @ kv) + x                 # [C, N]

Layout trick: the conv contraction (over input channels ci) produces
q in [c, n] layout (w^T stationary) AND k in [n, c] layout (x stationary)
with zero transposes; all other operands are host-prepped into layouts
where every DMA is large and contiguous-per-partition (15 input DMAs,
sized/ordered so the first conv tile's operands land first — the HWDGE
issue path serializes at ~650ns/DMA and descriptor-heavy transfers are
the classic real-HW cliff, so DMA count/shape is a first-order cost).

phi on the k half uses z = y + b + 1 accumulated directly in PSUM (the
conv bias + 1 enters as a rank-1 matmul appended to each accumulation
group), giving the 2-op form  max(z, min(exp(z-1), 1)):
one ACT exp (bias=-1) + one DVE scalar_tensor_tensor. The q half keeps
the 3-op form with the bias riding per-partition scalars; its min op
alternates DVE / ACT-relu per tile (relu shares the exp table set) so
neither helper engine exceeds PE's pace. Matmuls run in bf16 (f32 PSUM
accumulate) for pipelined LDWEIGHTS; dead warmup matmuls + a dummy exp
at t~1us start the PE p-state ramp and the Exp table load before real
operands arrive, and the tiny constant DMAs issue via SWDGE (Pool/Q7)
so they never occupy the serial ~625ns HWDGE slots ahead of the
start-critical weight/x transfers.
Phases run NT -> Q -> KV -> OUT: the ACT table switches Exp->Gelu
exactly once, and KV's pure-PE stretch drains the ACT/DVE backlog
right before the ACT-paced OUT chain. A single 4-slot PSUM pool of
[128,1024] double-bank tiles serves all phases, letting OUT fuse each
gelu across two banks. Residual add and the kv PSUM->SBUF copy run on
DVE (Pool's TensorTensor is ~1.1us/tile and would serialize the OUT
tail; ACT Copy would thrash the activation-table set). Output is bf16
(rel err ~4e-4 of the f32 path, tolerance is 2e-2) to halve the
output DMA bytes; the host casts back to f32.
"""

import ml_dtypes
import numpy as np

import concourse.mybir as mybir
import concourse.tile as tile
from concourse import bacc
from concourse.bass_utils import run_bass_kernel_spmd

F32 = mybir.dt.float32
BF16 = mybir.dt.bfloat16
AF = mybir.ActivationFunctionType
ALU = mybir.AluOpType

B, C, N = 8, 256, 4096
NCORES = 8
CT = C // 128        # 2 c-tiles (partition groups) per 256-channel dim
NJ = N // 512        # 8 column chunks of 512
NT = N // 128        # 32 n-tiles of 128
NP = N + 2           # x padded with one zero column on each side
KW = 3 * CT * 256    # one wt half: 6 blocks of [128, 256]

BF = ml_dtypes.bfloat16


def _build_nc():
    nc = bacc.Bacc("TRN2", target_bir_lowering=False, debug=False, num_devices=NCORES)

    xb_d = nc.declare_dram_parameter("xb", [CT, 128, NP], BF16, isOutput=False)
    vt_d = nc.declare_dram_parameter("vt", [128, NT * 256], BF16, isOutput=False)
    wt_d = nc.declare_dram_parameter("wt", [128, 2 * KW], BF16, isOutput=False)
    bq2_d = nc.declare_dram_parameter("bq2", [128, 13], F32, isOutput=False)
    out_d = nc.declare_dram_parameter("out", [C, N], BF16, isOutput=True)

    with tile.TileContext(nc) as tc:
        with (
            tc.tile_pool(name="persist", bufs=1) as per,
            tc.tile_pool(name="tmp", bufs=6) as tmp,
            tc.tile_pool(name="psum", bufs=4, space="PSUM") as ps,
        ):
            # ---- inputs: 11 large DMAs, start-critical ones first --------
            # (xb in three n-slices per ci so the first conv tiles can
            # start while the bulk is still in flight)
            wt_sb = per.tile([128, 2 * KW], BF16, tag="wt")
            xb_sb = [per.tile([128, NP], BF16, tag=f"xb{ci}", name=f"xb{ci}")
                     for ci in range(CT)]
            # tiny constants go via SWDGE (Pool/Q7) — a separate issue
            # path — so they don't occupy the serial 625ns/DMA HWDGE
            # slots ahead of the start-critical weight/x transfers
            bq2_sb = per.tile([128, 13], F32, tag="bq2")
            nc.gpsimd.dma_start(out=bq2_sb, in_=bq2_d[:, :])
            nc.sync.dma_start(out=wt_sb[:, 0:256], in_=wt_d[:, 0:256])
            nc.sync.dma_start(out=xb_sb[0][:, 0:514], in_=xb_d[0, :, 0:514])
            nc.sync.dma_start(out=wt_sb[:, 256:768], in_=wt_d[:, 256:768])
            nc.sync.dma_start(out=xb_sb[1][:, 0:514], in_=xb_d[1, :, 0:514])
            nc.sync.dma_start(out=wt_sb[:, 768:1536], in_=wt_d[:, 768:1536])
            for jc in range(3):
                a, b = 514 + jc * 512, 1026 + jc * 512
                for ci in range(CT):
                    nc.sync.dma_start(out=xb_sb[ci][:, a:b],
                                      in_=xb_d[ci, :, a:b])
            for ci in range(CT):
                nc.sync.dma_start(out=xb_sb[ci][:, 2050:NP],
                                  in_=xb_d[ci, :, 2050:NP])
            vt_sb = per.tile([128, NT * 256], BF16, tag="vt")
            nc.sync.dma_start(out=vt_sb, in_=vt_d[:, :])
            nc.sync.dma_start(out=wt_sb[:, KW:2 * KW], in_=wt_d[:, KW:2 * KW])

            def wk(t, cit):                    # k-half weights [128(ci), 256(co)]
                o = (cit * 3 + t) * 256
                return wt_sb[:, o:o + 256]

            def wq(t, cit):                    # q-half weights [128(ci), 256(co)]
                o = KW + (t * CT + cit) * 256
                return wt_sb[:, o:o + 256]

            # ---- persistent intermediates --------------------------------
            kT = per.tile([128, NT, 256], BF16, tag="kT")    # phi(k) in [n, c]
            qphi = [per.tile([128, N], BF16, tag=f"qphi{ct}", name=f"qphi{ct}")
                    for ct in range(CT)]
            kv_sb = per.tile([128, CT, 256], BF16, tag="kv")  # kv in [c, d]

            # ---- warmup: ramp the PE p-state while input DMAs land ------
            # (PE runs at half rate until ~3us of continuous busy; dead
            # matmuls on a memset scratch tile start the ramp at t~0.7us
            # instead of when the first real operands arrive)
            scratch = per.tile([128, 384], BF16, tag="warm")
            nc.vector.memset(scratch, 0.0)
            # dummy 1-elem exp: hoists the Exp table load (1.28us) to t~1us
            # instead of serializing it behind the first real exp's inputs
            dummy = tmp.tile([128, 1], F32, tag="dummy")
            nc.scalar.activation(dummy, scratch[:, 0:1], AF.Exp)
            wm_ps = ps.tile([128, 1024], F32, tag="bank", name="wm_ps")
            for w in range(12):
                nc.tensor.matmul(wm_ps[:, 0:256], scratch[:, 0:128],
                                 scratch[:, 128:384],
                                 start=(w == 0), stop=(w == 11))

            # ---- phase K: k = phi(conv_k + b) in [c, n] layout ----------
            # same structure as Q (per-partition bias on DVE/ACT scalars —
            # no rank-1 bias matmuls), then one XBAR DMA-transpose per
            # 128-channel tile produces k^T in [n, c] for the KV matmuls
            # on otherwise-idle DMA hardware (~261GB/s, fully overlapped)
            kphi = [per.tile([128, N], BF16, tag=f"kphi{ct}",
                             name=f"kphi{ct}") for ct in range(CT)]
            neg1 = bq2_sb[:, 4:5]
            for ct in range(CT):
                bk = bq2_sb[:, 7 + 3 * ct:8 + 3 * ct]
                bk1 = bq2_sb[:, 8 + 3 * ct:9 + 3 * ct]
                nbk = bq2_sb[:, 9 + 3 * ct:10 + 3 * ct]
                for j in range(NJ):
                    k_ps = ps.tile([128, 1024], F32, tag="bank",
                                   name="k_ps")
                    k_ps = k_ps[:, 0:512]
                    for ci in range(CT):
                        for t in range(3):
                            nc.tensor.matmul(
                                k_ps,
                                wk(t, ci)[:, ct * 128:(ct + 1) * 128],
                                xb_sb[ci][:, j * 512 + t:j * 512 + t + 512],
                                start=(ci == 0 and t == 0),
                                stop=(ci == CT - 1 and t == 2),
                            )
                    tmin = tmp.tile([128, 512], F32, tag="ktmin")
                    if j % 2 == 0:
                        nc.vector.tensor_scalar(
                            tmin, k_ps, bk, 0.0, ALU.add, ALU.min)
                        e = tmp.tile([128, 512], F32, tag="kte")
                        nc.scalar.activation(e, tmin, AF.Exp)
                    else:
                        nc.scalar.activation(tmin, k_ps, AF.Relu,
                                             bias=nbk, scale=neg1)
                        e = tmp.tile([128, 512], F32, tag="kte")
                        nc.scalar.activation(e, tmin, AF.Exp, scale=neg1)
                    nc.vector.scalar_tensor_tensor(
                        kphi[ct][:, j * 512:(j + 1) * 512],
                        k_ps, bk1, e, ALU.add, ALU.max)
                nc.sync.dma_start_transpose(
                    out=kT[:, :, ct * 128:(ct + 1) * 128],
                    in_=kphi[ct][:, :])

            # ---- phase Q: q = phi(conv_q + b) in [c, n] layout -----------
            for ct in range(CT):
                bq = bq2_sb[:, 2 * ct:2 * ct + 1]
                bq1 = bq2_sb[:, 2 * ct + 1:2 * ct + 2]
                nbq = bq2_sb[:, 5 + ct:6 + ct]
                for j in range(NJ):
                    q_ps = ps.tile([128, 1024], F32, tag="bank",
                                   name="q_ps")
                    q_ps = q_ps[:, 0:512]
                    for ci in range(CT):
                        for t in range(3):
                            nc.tensor.matmul(
                                q_ps,
                                wq(t, ci)[:, ct * 128:(ct + 1) * 128],
                                xb_sb[ci][:, j * 512 + t:j * 512 + t + 512],
                                start=(ci == 0 and t == 0),
                                stop=(ci == CT - 1 and t == 2),
                            )
                    # phi: min(y+b,0) -> exp -> (y + (b+1)) max e.  The
                    # min alternates DVE / ACT-relu (relu shares the exp
                    # table set) so neither engine exceeds PE's pace —
                    # all-DVE is DVE-bound, all-ACT is ACT-bound.
                    tmin = tmp.tile([128, 512], F32, tag="qtmin")
                    if j % 2 == 0:
                        nc.vector.tensor_scalar(
                            tmin, q_ps, bq, 0.0, ALU.add, ALU.min)
                        e = tmp.tile([128, 512], F32, tag="qte")
                        nc.scalar.activation(e, tmin, AF.Exp)
                    else:
                        nc.scalar.activation(tmin, q_ps, AF.Relu,
                                             bias=nbq, scale=neg1)
                        e = tmp.tile([128, 512], F32, tag="qte")
                        nc.scalar.activation(e, tmin, AF.Exp, scale=neg1)
                    nc.vector.scalar_tensor_tensor(
                        qphi[ct][:, j * 512:(j + 1) * 512],
                        q_ps, bq1, e, ALU.add, ALU.max)

            # ---- phase KV: kv[c, d] = sum_n k^T[n, c] v^T[n, d] ----------
            for ch in range(CT):
                kv_ps = ps.tile([128, 1024], F32, tag="bank", name="kv_ps")
                kv_ps = kv_ps[:, 0:256]
                for i in range(NT):
                    nc.tensor.matmul(
                        kv_ps,
                        kT[:, i, ch * 128:(ch + 1) * 128],
                        vt_sb[:, i * 256:(i + 1) * 256],
                        start=(i == 0),
                        stop=(i == NT - 1),
                    )
                nc.vector.tensor_copy(kv_sb[:, ch, :], kv_ps)

            # ---- phase OUT: out[d, n] = gelu(sum_c kv[c, d] q[c, n]) + x -
            # pairs of j-chunks share one residual add + one output DMA so
            # the HWDGE issue rate (625ns/DMA) stays ahead of ACT's gelu
            # rate and the tail drains fast
            for dt in range(CT):
                for jj in range(NJ // 2):
                    last = (dt == CT - 1 and jj == NJ // 2 - 1)
                    o_ps = ps.tile([128, 1024], F32, tag="bank",
                                   name="o_ps")
                    for h in range(2):
                        j = 2 * jj + h
                        for ch in range(CT):
                            nc.tensor.matmul(
                                o_ps[:, h * 512:(h + 1) * 512],
                                kv_sb[:, ch, dt * 128:(dt + 1) * 128],
                                qphi[ch][:, j * 512:(j + 1) * 512],
                                start=(ch == 0),
                                stop=(ch == CT - 1),
                            )
                    if not last:
                        g = tmp.tile([128, 1024], BF16, tag="og")
                        nc.scalar.activation(g, o_ps, AF.Gelu)
                        o = tmp.tile([128, 1024], BF16, tag="oo")
                        nc.vector.tensor_add(
                            o, g,
                            xb_sb[dt][:, 1 + jj * 1024:1 + (jj + 1) * 1024])
                        nc.sync.dma_start(
                            out=out_d[dt * 128:(dt + 1) * 128,
                                      jj * 1024:(jj + 1) * 1024],
                            in_=o,
                        )
                    else:
                        # final group drains per-512 so the tail chain ends
                        # on a half-size gelu/add/DMA
                        for h in range(2):
                            j = 2 * jj + h
                            gh = tmp.tile([128, 512], BF16, tag="ogh")
                            nc.scalar.activation(
                                gh, o_ps[:, h * 512:(h + 1) * 512], AF.Gelu)
                            oh = tmp.tile([128, 512], BF16, tag="ooh")
                            nc.vector.tensor_add(
                                oh, gh,
                                xb_sb[dt][:, 1 + j * 512:1 + (j + 1) * 512])
                            nc.sync.dma_start(
                                out=out_d[dt * 128:(dt + 1) * 128,
                                          j * 512:(j + 1) * 512],
                                in_=oh,
                            )

    nc.compile()
    return nc


_NC_CACHE = None


def _get_nc():
    global _NC_CACHE
    if _NC_CACHE is None:
        _NC_CACHE = _build_nc()
    return _NC_CACHE


def _prep(x, conv_w, conv_b):
    x = np.asarray(x, dtype=np.float32)
    conv_w = np.asarray(conv_w, dtype=np.float32)
    conv_b = np.asarray(conv_b, dtype=np.float32)
    xb = np.zeros((B, CT, 128, NP), dtype=BF)
    xb[:, :, :, 1:N + 1] = x.reshape(B, CT, 128, N).astype(BF)
    # vt[b, p, i*256 + d] = x[b, d, i*128 + p]
    xt = x.transpose(0, 2, 1)                              # [B, N, C]
    vt = np.ascontiguousarray(
        xt.reshape(B, NT, 128, C).transpose(0, 2, 1, 3)
    ).reshape(B, 128, NT * C).astype(BF)
    # wt[ci, half, (t*CT + cit)*256 + co'] = conv_w[half*256 + co', cit*128 + ci, t]
    w4 = (conv_w.transpose(1, 2, 0)                        # [cin, t, co]
          .reshape(CT, 128, 3, 2 * C)                      # [cit, ci, t, co]
          .transpose(1, 2, 0, 3))                          # [ci, t, cit, co]
    wt = np.concatenate(
        [w4[..., C:2 * C].transpose(0, 2, 1, 3)            # k half, cit-major
         .reshape(128, KW),
         w4[..., 0:C].reshape(128, KW)],                   # q half, t-major
        axis=1).astype(BF)
    bq2 = np.empty((128, 13), dtype=np.float32)
    for ct in range(CT):
        bq2[:, 2 * ct] = conv_b[ct * 128:(ct + 1) * 128]
        bq2[:, 2 * ct + 1] = conv_b[ct * 128:(ct + 1) * 128] + 1.0
        bq2[:, 5 + ct] = -conv_b[ct * 128:(ct + 1) * 128]
        bk = conv_b[C + ct * 128:C + (ct + 1) * 128]
        bq2[:, 7 + 3 * ct] = bk
        bq2[:, 8 + 3 * ct] = bk + 1.0
        bq2[:, 9 + 3 * ct] = -bk
    bq2[:, 4] = -1.0
    return xb, vt, wt, bq2


def make_in_maps(x, conv_w, conv_b):
    xb, vt, wt, bq2 = _prep(x, conv_w, conv_b)
    return [
        {"xb": xb[b], "vt": vt[b], "wt": wt, "bq2": bq2}
        for b in range(B)
    ]


def kernel(x: np.ndarray, conv_w: np.ndarray, conv_b: np.ndarray) -> np.ndarray:
    nc = _get_nc()
    in_maps = make_in_maps(x, conv_w, conv_b)
    res = run_bass_kernel_spmd(nc, in_maps, core_ids=list(range(NCORES)))
    return np.stack(
        [res.results[b]["out"].astype(np.float32) for b in range(B)], axis=0)


# revision 66
# speedup vs baseline: 1.3080x; 1.2981x over previous
"""Trainium2 Bass kernel for nn_AttentionLayer (conv1d -> linear attention -> gelu + residual).

Full inputs:  x [8, 256, 4096] f32, conv_w [512, 256, 3] f32, conv_b [512] f32
Full output:  [8, 256, 4096] f32

Sharding: pure data-parallel over batch B=8 -> 8 NeuronCores, one batch each.
No collectives needed.

Per-core math (C=256, N=4096, one batch):
  y    = conv1d(x, w, pad=1) + b          # [2C, N]
  q    = phi(y[:C]),  k = phi(y[C:])      # phi = elu+1
  v    = x^T                              # [N, C]
  kv   = sum_n phi(k)[n,:] (x) v[n,:]     # [C, C]
  out  = gelu(q @ kv) + x                 # [C, N]

Layout trick: both conv halves are computed in [c, n] layout (weights
stationary, x moving, conv bias riding per-partition DVE/ACT scalars —
no rank-1 bias matmuls), and k^T in [n, c] for the KV contraction is
produced by two XBAR DMA-transposes (~261GB/s) on otherwise-idle DMA
hardware, fully overlapped with the Q phase; v^T = x^T comes from two
more XBAR transposes of the already-loaded x, replacing the 2.1MB
host-transposed vt input entirely (total input traffic: 2.9MB). All
operands are host-prepped into layouts where every DMA is large and
contiguous-per-partition, sized/ordered so the first conv tile's
operands land first (the HWDGE issue path serializes at ~650ns/DMA and
descriptor-heavy transfers are the classic real-HW cliff); tiny
constants issue via SWDGE (Pool/Q7) to stay off the HWDGE slots.
The K phase iterates j-outer so x-chunk demand stays behind the DMA
supply rate; phi's min op alternates DVE / ACT-relu per j (relu shares
the exp table set) so neither helper engine exceeds PE's pace.
Matmuls run in bf16 (f32 PSUM accumulate) for pipelined LDWEIGHTS;
dead warmup matmuls + a dummy exp at t~1us start the PE p-state ramp
and the Exp table load before real operands arrive.
Phases run K -> Q -> KV -> OUT: the ACT table switches Exp->Gelu
exactly once, and KV's pure-PE stretch drains the ACT/DVE backlog
right before the ACT-paced OUT chain. A single 4-slot PSUM pool of
[128,1024] double-bank tiles serves all phases, letting OUT fuse each
gelu across two banks. Residual add and the kv PSUM->SBUF copy run on
DVE (Pool's TensorTensor is ~1.1us/tile and would serialize the OUT
tail; ACT Copy would thrash the activation-table set). Output is bf16
(rel err ~4e-4 of the f32 path, tolerance is 2e-2) to halve the
output DMA bytes; the host casts back to f32.
"""

import ml_dtypes
import numpy as np

import concourse.mybir as mybir
import concourse.tile as tile
from concourse import bacc
from concourse.bass_utils import run_bass_kernel_spmd

F32 = mybir.dt.float32
BF16 = mybir.dt.bfloat16
AF = mybir.ActivationFunctionType
ALU = mybir.AluOpType

B, C, N = 8, 256, 4096
NCORES = 8
CT = C // 128        # 2 c-tiles (partition groups) per 256-channel dim
NJ = N // 512        # 8 column chunks of 512
NT = N // 128        # 32 n-tiles of 128
NP = N + 2           # x padded with one zero column on each side
KW = 3 * CT * 256    # one wt half: 6 blocks of [128, 256]

BF = ml_dtypes.bfloat16


def _build_nc():
    nc = bacc.Bacc("TRN2", target_bir_lowering=False, debug=False, num_devices=NCORES)

    xb_d = nc.declare_dram_parameter("xb", [CT, 128, NP], BF16, isOutput=False)
    wt_d = nc.declare_dram_parameter("wt", [128, 2 * KW], BF16, isOutput=False)
    bq2_d = nc.declare_dram_parameter("bq2", [128, 13], F32, isOutput=False)
    out_d = nc.declare_dram_parameter("out", [C, N], BF16, isOutput=True)

    with tile.TileContext(nc) as tc:
        with (
            tc.tile_pool(name="persist", bufs=1) as per,
            tc.tile_pool(name="tmp", bufs=6) as tmp,
            tc.tile_pool(name="psum", bufs=4, space="PSUM") as ps,
        ):
            # ---- inputs: 11 large DMAs, start-critical ones first --------
            # (xb in three n-slices per ci so the first conv tiles can
            # start while the bulk is still in flight)
            wt_sb = per.tile([128, 2 * KW], BF16, tag="wt")
            xb_sb = [per.tile([128, NP], BF16, tag=f"xb{ci}", name=f"xb{ci}")
                     for ci in range(CT)]
            # tiny constants go via SWDGE (Pool/Q7) — a separate issue
            # path — so they don't occupy the serial 625ns/DMA HWDGE
            # slots ahead of the start-critical weight/x transfers
            bq2_sb = per.tile([128, 13], F32, tag="bq2")
            nc.gpsimd.dma_start(out=bq2_sb, in_=bq2_d[:, :])
            nc.sync.dma_start(out=wt_sb[:, 0:256], in_=wt_d[:, 0:256])
            nc.sync.dma_start(out=xb_sb[0][:, 0:514], in_=xb_d[0, :, 0:514])
            nc.sync.dma_start(out=wt_sb[:, 256:768], in_=wt_d[:, 256:768])
            nc.sync.dma_start(out=xb_sb[1][:, 0:514], in_=xb_d[1, :, 0:514])
            nc.sync.dma_start(out=wt_sb[:, 768:1536], in_=wt_d[:, 768:1536])
            for jc in range(3):
                a, b = 514 + jc * 512, 1026 + jc * 512
                for ci in range(CT):
                    nc.sync.dma_start(out=xb_sb[ci][:, a:b],
                                      in_=xb_d[ci, :, a:b])
            for ci in range(CT):
                nc.sync.dma_start(out=xb_sb[ci][:, 2050:NP],
                                  in_=xb_d[ci, :, 2050:NP])
            nc.sync.dma_start(out=wt_sb[:, KW:2 * KW], in_=wt_d[:, KW:2 * KW])
            # v^T = x^T on-device: two XBAR transposes of the already-loaded
            # x replace the host-transposed 2.1MB vt input entirely
            vt_sb = per.tile([128, NT, 256], BF16, tag="vt")
            for ci in range(CT):
                nc.sync.dma_start_transpose(
                    out=vt_sb[:, :, ci * 128:(ci + 1) * 128],
                    in_=xb_sb[ci][:, 1:N + 1])

            def wk(t, cit):                    # k-half weights [128(ci), 256(co)]
                o = (cit * 3 + t) * 256
                return wt_sb[:, o:o + 256]

            def wq(t, cit):                    # q-half weights [128(ci), 256(co)]
                o = KW + (t * CT + cit) * 256
                return wt_sb[:, o:o + 256]

            # ---- persistent intermediates --------------------------------
            kT = per.tile([128, NT, 256], BF16, tag="kT")    # phi(k) in [n, c]
            qphi = [per.tile([128, N], BF16, tag=f"qphi{ct}", name=f"qphi{ct}")
                    for ct in range(CT)]
            kv_sb = per.tile([128, CT, 256], BF16, tag="kv")  # kv in [c, d]

            # ---- warmup: ramp the PE p-state while input DMAs land ------
            # (PE runs at half rate until ~3us of continuous busy; dead
            # matmuls on a memset scratch tile start the ramp at t~0.7us
            # instead of when the first real operands arrive)
            scratch = per.tile([128, 384], BF16, tag="warm")
            nc.vector.memset(scratch, 0.0)
            # dummy 1-elem exp: hoists the Exp table load (1.28us) to t~1us
            # instead of serializing it behind the first real exp's inputs
            dummy = tmp.tile([128, 1], F32, tag="dummy")
            nc.scalar.activation(dummy, scratch[:, 0:1], AF.Exp)
            wm_ps = ps.tile([128, 1024], F32, tag="bank", name="wm_ps")
            for w in range(12):
                nc.tensor.matmul(wm_ps[:, 0:256], scratch[:, 0:128],
                                 scratch[:, 128:384],
                                 start=(w == 0), stop=(w == 11))

            # ---- phase K: k = phi(conv_k + b) in [c, n] layout ----------
            # same structure as Q (per-partition bias on DVE/ACT scalars —
            # no rank-1 bias matmuls), then one XBAR DMA-transpose per
            # 128-channel tile produces k^T in [n, c] for the KV matmuls
            # on otherwise-idle DMA hardware (~261GB/s, fully overlapped)
            kphi = [per.tile([128, N], BF16, tag=f"kphi{ct}",
                             name=f"kphi{ct}") for ct in range(CT)]
            neg1 = bq2_sb[:, 4:5]
            for j in range(NJ):
                for ct in range(CT):
                    bk = bq2_sb[:, 7 + 3 * ct:8 + 3 * ct]
                    bk1 = bq2_sb[:, 8 + 3 * ct:9 + 3 * ct]
                    nbk = bq2_sb[:, 9 + 3 * ct:10 + 3 * ct]
                    k_ps = ps.tile([128, 1024], F32, tag="bank",
                                   name="k_ps")
                    k_ps = k_ps[:, 0:512]
                    for ci in range(CT):
                        for t in range(3):
                            nc.tensor.matmul(
                                k_ps,
                                wk(t, ci)[:, ct * 128:(ct + 1) * 128],
                                xb_sb[ci][:, j * 512 + t:j * 512 + t + 512],
                                start=(ci == 0 and t == 0),
                                stop=(ci == CT - 1 and t == 2),
                            )
                    tmin = tmp.tile([128, 512], F32, tag="ktmin")
                    if j % 2 == 0:
                        nc.vector.tensor_scalar(
                            tmin, k_ps, bk, 0.0, ALU.add, ALU.min)
                        e = tmp.tile([128, 512], F32, tag="kte")
                        nc.scalar.activation(e, tmin, AF.Exp)
                    else:
                        nc.scalar.activation(tmin, k_ps, AF.Relu,
                                             bias=nbk, scale=neg1)
                        e = tmp.tile([128, 512], F32, tag="kte")
                        nc.scalar.activation(e, tmin, AF.Exp, scale=neg1)
                    nc.vector.scalar_tensor_tensor(
                        kphi[ct][:, j * 512:(j + 1) * 512],
                        k_ps, bk1, e, ALU.add, ALU.max)
            for ct in range(CT):
                nc.sync.dma_start_transpose(
                    out=kT[:, :, ct * 128:(ct + 1) * 128],
                    in_=kphi[ct][:, :])

            # ---- phase Q: q = phi(conv_q + b) in [c, n] layout -----------
            for ct in range(CT):
                bq = bq2_sb[:, 2 * ct:2 * ct + 1]
                bq1 = bq2_sb[:, 2 * ct + 1:2 * ct + 2]
                nbq = bq2_sb[:, 5 + ct:6 + ct]
                for j in range(NJ):
                    q_ps = ps.tile([128, 1024], F32, tag="bank",
                                   name="q_ps")
                    q_ps = q_ps[:, 0:512]
                    for ci in range(CT):
                        for t in range(3):
                            nc.tensor.matmul(
                                q_ps,
                                wq(t, ci)[:, ct * 128:(ct + 1) * 128],
                                xb_sb[ci][:, j * 512 + t:j * 512 + t + 512],
                                start=(ci == 0 and t == 0),
                                stop=(ci == CT - 1 and t == 2),
                            )
                    # phi: min(y+b,0) -> exp -> (y + (b+1)) max e.  The
                    # min alternates DVE / ACT-relu (relu shares the exp
                    # table set) so neither engine exceeds PE's pace —
                    # all-DVE is DVE-bound, all-ACT is ACT-bound.
                    tmin = tmp.tile([128, 512], F32, tag="qtmin")
                    if j % 2 == 0:
                        nc.vector.tensor_scalar(
                            tmin, q_ps, bq, 0.0, ALU.add, ALU.min)
                        e = tmp.tile([128, 512], F32, tag="qte")
                        nc.scalar.activation(e, tmin, AF.Exp)
                    else:
                        nc.scalar.activation(tmin, q_ps, AF.Relu,
                                             bias=nbq, scale=neg1)
                        e = tmp.tile([128, 512], F32, tag="qte")
                        nc.scalar.activation(e, tmin, AF.Exp, scale=neg1)
                    nc.vector.scalar_tensor_tensor(
                        qphi[ct][:, j * 512:(j + 1) * 512],
                        q_ps, bq1, e, ALU.add, ALU.max)

            # ---- phase KV: kv[c, d] = sum_n k^T[n, c] v^T[n, d] ----------
            for ch in range(CT):
                kv_ps = ps.tile([128, 1024], F32, tag="bank", name="kv_ps")
                kv_ps = kv_ps[:, 0:256]
                for i in range(NT):
                    nc.tensor.matmul(
                        kv_ps,
                        kT[:, i, ch * 128:(ch + 1) * 128],
                        vt_sb[:, i, :],
                        start=(i == 0),
                        stop=(i == NT - 1),
                    )
                nc.vector.tensor_copy(kv_sb[:, ch, :], kv_ps)

            # ---- phase OUT: out[d, n] = gelu(sum_c kv[c, d] q[c, n]) + x -
            # pairs of j-chunks share one residual add + one output DMA so
            # the HWDGE issue rate (625ns/DMA) stays ahead of ACT's gelu
            # rate and the tail drains fast
            for dt in range(CT):
                for jj in range(NJ // 2):
                    last = (dt == CT - 1 and jj == NJ // 2 - 1)
                    o_ps = ps.tile([128, 1024], F32, tag="bank",
                                   name="o_ps")
                    for h in range(2):
                        j = 2 * jj + h
                        for ch in range(CT):
                            nc.tensor.matmul(
                                o_ps[:, h * 512:(h + 1) * 512],
                                kv_sb[:, ch, dt * 128:(dt + 1) * 128],
                                qphi[ch][:, j * 512:(j + 1) * 512],
                                start=(ch == 0),
                                stop=(ch == CT - 1),
                            )
                    if not last:
                        g = tmp.tile([128, 1024], BF16, tag="og")
                        nc.scalar.activation(g, o_ps, AF.Gelu)
                        o = tmp.tile([128, 1024], BF16, tag="oo")
                        nc.vector.tensor_add(
                            o, g,
                            xb_sb[dt][:, 1 + jj * 1024:1 + (jj + 1) * 1024])
                        nc.sync.dma_start(
                            out=out_d[dt * 128:(dt + 1) * 128,
                                      jj * 1024:(jj + 1) * 1024],
                            in_=o,
                        )
                    else:
                        # final group drains per-512 so the tail chain ends
                        # on a half-size gelu/add/DMA
                        for h in range(2):
                            j = 2 * jj + h
                            gh = tmp.tile([128, 512], BF16, tag="ogh")
                            nc.scalar.activation(
                                gh, o_ps[:, h * 512:(h + 1) * 512], AF.Gelu)
                            oh = tmp.tile([128, 512], BF16, tag="ooh")
                            nc.vector.tensor_add(
                                oh, gh,
                                xb_sb[dt][:, 1 + j * 512:1 + (j + 1) * 512])
                            nc.sync.dma_start(
                                out=out_d[dt * 128:(dt + 1) * 128,
                                          j * 512:(j + 1) * 512],
                                in_=oh,
                            )

    nc.compile()
    return nc


_NC_CACHE = None


def _get_nc():
    global _NC_CACHE
    if _NC_CACHE is None:
        _NC_CACHE = _build_nc()
    return _NC_CACHE


def _prep(x, conv_w, conv_b):
    x = np.asarray(x, dtype=np.float32)
    conv_w = np.asarray(conv_w, dtype=np.float32)
    conv_b = np.asarray(conv_b, dtype=np.float32)
    xb = np.zeros((B, CT, 128, NP), dtype=BF)
    xb[:, :, :, 1:N + 1] = x.reshape(B, CT, 128, N).astype(BF)
    # wt[ci, half, (t*CT + cit)*256 + co'] = conv_w[half*256 + co', cit*128 + ci, t]
    w4 = (conv_w.transpose(1, 2, 0)                        # [cin, t, co]
          .reshape(CT, 128, 3, 2 * C)                      # [cit, ci, t, co]
          .transpose(1, 2, 0, 3))                          # [ci, t, cit, co]
    wt = np.concatenate(
        [w4[..., C:2 * C].transpose(0, 2, 1, 3)            # k half, cit-major
         .reshape(128, KW),
         w4[..., 0:C].reshape(128, KW)],                   # q half, t-major
        axis=1).astype(BF)
    bq2 = np.empty((128, 13), dtype=np.float32)
    for ct in range(CT):
        bq2[:, 2 * ct] = conv_b[ct * 128:(ct + 1) * 128]
        bq2[:, 2 * ct + 1] = conv_b[ct * 128:(ct + 1) * 128] + 1.0
        bq2[:, 5 + ct] = -conv_b[ct * 128:(ct + 1) * 128]
        bk = conv_b[C + ct * 128:C + (ct + 1) * 128]
        bq2[:, 7 + 3 * ct] = bk
        bq2[:, 8 + 3 * ct] = bk + 1.0
        bq2[:, 9 + 3 * ct] = -bk
    bq2[:, 4] = -1.0
    return xb, wt, bq2


def make_in_maps(x, conv_w, conv_b):
    xb, wt, bq2 = _prep(x, conv_w, conv_b)
    return [
        {"xb": xb[b], "wt": wt, "bq2": bq2}
        for b in range(B)
    ]


def kernel(x: np.ndarray, conv_w: np.ndarray, conv_b: np.ndarray) -> np.ndarray:
    nc = _get_nc()
    in_maps = make_in_maps(x, conv_w, conv_b)
    res = run_bass_kernel_spmd(nc, in_maps, core_ids=list(range(NCORES)))
    return np.stack(
        [res.results[b]["out"].astype(np.float32) for b in range(B)], axis=0)
